# revision 59
# baseline (speedup 1.0000x reference)
"""DepthLSSTransform Trainium kernel: 3 SPMD launches over 8 NeuronCores.

Launch A: per-camera conv pipeline (dtransform + depthnet + softmax) on
          24-row bands (one 16-row + one 8-row segment per core).
Launch B: bev_pool segment-sum via one-hot matmuls over a host-built
          virtual-window schedule (sorted-by-voxel points).
Launch C: BEV downsample convs, spatially sharded.
Host: geometry/voxel indices, scheduling, gathers, folds (orchestration).
"""
import numpy as np
import ml_dtypes

import concourse.bass as bass
import concourse.tile as tile
from concourse import bacc, mybir
from concourse.bass_utils import run_bass_kernel_spmd

dt = mybir.dt
bf16 = ml_dtypes.bfloat16

# ---- problem constants (hardcoded per contract) ----
B, N = 1, 6
CIN, CIMG, DD = 256, 80, 59
FH, FW, IH, IW = 32, 88, 256, 704
XY0, DXY, NX = -54.0, 0.3, 360
Z0, DZ, NZ = -10.0, 20.0, 1
NPTS = N * DD * FH * FW
NPIX = N * FH * FW
NCORES = 8
QV = 4                      # chunks of 128 points per virtual window

# per-core segments: (camera, h0) for seg A (16 rows) and seg B (8 rows)
SEG_A = [(0, 0), (1, 0), (1, 16), (2, 16), (3, 0), (4, 0), (4, 16), (5, 16)]
SEG_B = [(0, 16), (0, 24), (2, 0), (2, 8), (3, 16), (3, 24), (5, 0), (5, 8)]
# band pixel ranges in global row order (row = n*32 + h)
ROWS_OF_CORE = [[(SEG_A[c][0] * FH + SEG_A[c][1] + r) for r in range(16)] +
                [(SEG_B[c][0] * FH + SEG_B[c][1] + r) for r in range(8)]
                for c in range(NCORES)]

# segment geometry: rows16 segment: d rows [8h0-34, 8h0+158) (192), dt2 out
# rows [2h0-8, 2h0+39) (47), dt3 [h0-3, h0+19) (22), dn1 [h0-1, h0+17) (18)
SEGS = [dict(nout=16, nd=192, nq=48, nt2=47, nt3=22, nn1=18),
        dict(nout=8, nd=128, nq=32, nt2=31, nt3=14, nn1=10)]


def _seg_ranges(h0, S):
    return dict(d0=8 * h0 - 34, q0=2 * h0 - 8, t0=h0 - 3, r0=h0 - 1, o0=h0)


# ---------------------------------------------------------------- launch A
def build_launch_a(psum_bufs=4, work_bufs=3):
    nc = bacc.Bacc("TRN2", target_bir_lowering=False, debug=False,
                   num_devices=NCORES)
    AP = {}

    def inp(name, shape, dtype=dt.bfloat16):
        AP[name] = nc.dram_tensor(name, shape, dtype, kind="ExternalInput").ap()
        return AP[name]

    # per segment inputs (s = 0: 16-row, 1: 8-row)
    out_depth, out_feat = {}, {}
    for s, S in enumerate(SEGS):
        inp(f"dph{s}", [128, S["nq"], 177])             # sentinel-padded d
        inp(f"masks{s}", [128, S["nt2"] + S["nt3"] + S["nn1"]])
        inp(f"xseg{s}", [CIN, S["nt3"], 92])            # x_img slice, 92-padded
        pcs = (S["nout"] * FW + 127) // 128
        out_depth[s] = nc.dram_tensor(f"out_depth{s}", [128, pcs, DD],
                                      dt.bfloat16, kind="ExternalOutput").ap()
        out_feat[s] = nc.dram_tensor(f"out_feat{s}", [128, pcs, CIMG],
                                     dt.bfloat16, kind="ExternalOutput").ap()
    # packed f32 constants: [alpha, beta, s_dt2, t_dt2, s_dt3, t_dt3,
    #  s_dn1(2), t_dn1(2), s_dn2(2), t_dn2(2), b_dn3(139)] -> [128, 153]
    inp("consts", [128, 153], dt.float32)
    # conv weights (host-prepped layouts)
    inp("w_dt2", [4, 128, 32])                          # groups (dky,dmx)
    inp("w_dt3", [9, 128, 64])
    inp("w_dn1", [9, 3, 128, 256])                      # tap, icchunk(128,128,64pad) -> 256
    inp("w_dn2", [9, 2, 128, 256])
    inp("w_dn3", [2, 128, 139])
    inp("w_dn1d", [3, 128, 256])                        # dtc ky{0,1}-packed
    inp("bias_dn3", [1, 139])                           # K=1 bias row

    RELU = mybir.ActivationFunctionType.Relu
    with tile.TileContext(nc) as tc:
        with tc.tile_pool(name="const", bufs=1) as cpool, \
             tc.tile_pool(name="work", bufs=work_bufs) as wpool, \
             tc.tile_pool(name="big", bufs=1) as bpool, \
             tc.tile_pool(name="psum", bufs=psum_bufs, space="PSUM") as ppool:
            # ---- load packed constants in one DMA ----
            cts = cpool.tile([128, 153], dt.float32, name="cts")
            nc.sync.dma_start(out=cts[:], in_=AP["consts"])
            ct = {"dt1_alpha": cts[:, 0:1], "dt1_beta": cts[:, 1:2],
                  "s_dt2": cts[:, 2:3], "t_dt2": cts[:, 3:4],
                  "s_dt3": cts[:, 4:5], "t_dt3": cts[:, 5:6],
                  "s_dn1": cts[:, 6:8], "t_dn1": cts[:, 8:10],
                  "s_dn2": cts[:, 10:12], "t_dn2": cts[:, 12:14]}
            one1 = cpool.tile([1, 128], dt.bfloat16, name="one1")
            nc.vector.memset(one1[:], 1.0)
            wt = {}

            def load_w(nm, pat):
                sh = list(AP[nm].shape)
                wt[nm] = cpool.tile([sh[-2], int(np.prod(sh[:-2])), sh[-1]],
                                    dt.bfloat16, tag=nm, name=f'wt_{nm}')
                nc.sync.dma_start(out=wt[nm][:], in_=AP[nm].rearrange(pat))

            # DMA issue order = need order: dt1/dt2 inputs first, big dn
            # weights (needed ~40us in) last
            load_w("w_dt2", "g p o -> p g o")

            SG = list(enumerate(SEGS))
            T = {}                          # per-seg tiles

            # ======== dt1: relu(alpha*d + beta); pads are host sentinels ====
            for s, S in SG:
                nq = S["nq"]
                dph = []
                bnds = [0, 6] + list(range(18, nq, 12)) + [nq]
                T[s] = {}
                for ci in range(len(bnds) - 1):
                    qq = bnds[ci]
                    nqq = min(bnds[ci + 1] + 1, nq) - qq
                    dpc = bpool.tile([128, 13, 177], dt.bfloat16,
                                     tag=f"dph{s}_{ci}", name=f"dph{s}_{ci}")
                    nc.sync.dma_start(out=dpc[:, 0:nqq, :],
                                      in_=AP[f"dph{s}"][:, qq:qq + nqq, :])
                    dph.append(dpc)
                T[s] = dict(bnds=bnds, dph=dph)
                mall = wpool.tile([128, S["nt2"] + S["nt3"] + S["nn1"]],
                                  dt.bfloat16, tag=f"msk{s}", name="mall")
                nc.sync.dma_start(out=mall[:], in_=AP[f"masks{s}"])
                T[s]["mall"] = mall
            load_w("w_dt3", "g p o -> p g o")
            for s, S in SG:
                xs = []
                for g in range(2):
                    xt = bpool.tile([128, S["nt3"], 92], dt.bfloat16,
                                    tag=f"x{g}_{s}", name=f"xseg_t{g}")
                    nc.sync.dma_start(out=xt[:],
                                      in_=AP[f"xseg{s}"][g * 128:(g + 1) * 128])
                    xs.append(xt)
                T[s]["xs"] = xs
            load_w("w_dn1", "t i p o -> p (t i) o")
            load_w("w_dn1d", "g p o -> p g o")
            load_w("w_dn2", "t i p o -> p (t i) o")
            load_w("w_dn3", "g p o -> p g o")
            b3t = cpool.tile([1, 139], dt.bfloat16, name="b3t")
            nc.sync.dma_start(out=b3t[:], in_=AP["bias_dn3"])
            for s, S in SG:
                nq = S["nq"]
                dph = T[s]["dph"]
                # t1 split into per-chunk tiles mirroring dph chunks (each
                # covers [bnds[ci], bnds[ci+1]] incl halo row) so dt2 groups
                # start as soon as their chunk's dt1 op completes
                bnds = T[s]["bnds"]
                t1s = []
                for ci in range(len(bnds) - 1):
                    qq = bnds[ci]
                    nqq = min(bnds[ci + 1] + 1, nq) - qq
                    t1 = bpool.tile([128, 13, 177], dt.bfloat16,
                                    tag=f"t1{s}_{ci}", name=f"t1{s}_{ci}")
                    src = dph[ci][:, 0:nqq, :]
                    if True:
                        nc.vector.tensor_scalar(out=t1[:, 0:nqq, :],
                                                in0=src,
                                                scalar1=ct["dt1_alpha"][:, 0:1],
                                                scalar2=ct["dt1_beta"][:, 0:1],
                                                op0=mybir.AluOpType.mult,
                                                op1=mybir.AluOpType.add)
                        nc.vector.tensor_scalar(out=t1[:, 0:nqq, :],
                                                in0=t1[:, 0:nqq, :],
                                                scalar1=0.0, scalar2=None,
                                                op0=mybir.AluOpType.max)
                    else:
                        nc.scalar.activation(t1[:, 0:nqq, :], src, RELU,
                                             bias=ct["dt1_beta"][:, 0:1],
                                             scale=ct["dt1_alpha"][:, 0:1])
                    t1s.append(t1)
                T[s]["t1"] = t1s

            # ======== dt2: 5x5 s4 conv as 4 phase matmuls ========
            for s, S in SG:
                nq, nt2 = S["nq"], S["nt2"]
                t1, mall = T[s]["t1"], T[s]["mall"]
                o2 = bpool.tile([32, nt2 + 1, 180], dt.bfloat16, tag=f"o2{s}",
                                name=f"o2{s}")
                nc.vector.memset(o2[:, :, 0:1], 0.0)
                nc.vector.memset(o2[:, :, 89:91], 0.0)
                nc.vector.memset(o2[:, :, 179:180], 0.0)
                nc.vector.memset(o2[:, nt2:nt2 + 1, :], 0.0)
                m2 = bass.AP(mall.tensor, mall.offset, [mall.ap[0], [1, nt2]])
                RPP2 = 2
                for q0 in range(0, nt2, RPP2):
                    nr = min(RPP2, nt2 - q0)
                    ps = ppool.tile([32, nr, 176], dt.float32, tag=f"ps{s}",
                                    name="ps2")
                    gi = 0
                    import bisect
                    ci = bisect.bisect_right(T[s]["bnds"], q0) - 1
                    for dky in range(2):
                        t1c = t1[ci]
                        row = q0 + dky - T[s]["bnds"][ci]
                        for dmx in range(2):
                            g = dky * 2 + dmx
                            rhs = bass.AP(
                                t1c.tensor, t1c.offset + row * 177 + dmx,
                                [t1c.ap[0], [177, nr], [1, 176]])
                            nc.tensor.matmul(ps[:], wt["w_dt2"][:, g, :], rhs,
                                             start=(gi == 0), stop=(gi == 3))
                            gi += 1
                    ev = wpool.tile([32, nr, 176], dt.bfloat16, tag=f"ev2{s}")
                    nc.scalar.activation(ev[:], ps[:], RELU,
                                         bias=ct["t_dt2"][0:32, 0:1],
                                         scale=ct["s_dt2"][0:32, 0:1])
                    mbb = bass.AP(m2.tensor, m2.offset + q0,
                                  [[m2.ap[0][0], 32], [1, nr], [0, 176]])
                    # write col c at (c%2)*90 + c//2 + 1  (phase-split layout)
                    o2dst = bass.AP(o2.tensor, o2.offset + q0 * 180 + 1,
                                    [[o2.ap[0][0], 32], [180, nr],
                                     [1, 88], [90, 2]])
                    nc.vector.tensor_tensor(out=o2dst, in0=ev[:], in1=mbb,
                                            op=mybir.AluOpType.mult)
                T[s]["o2"] = o2

            # ======== dt3: 5x5 s2 conv; ph3 gathered from o2 (SBUF->SBUF) ====
            for s, S in SG:
                nt2, nt3 = S["nt2"], S["nt3"]
                o2 = T[s]["o2"]
                nry3 = nt3 + 2
                ph3 = bpool.tile([128, nry3, 90], dt.bfloat16, tag=f"ph3{s}",
                                 name=f"ph3{s}")
                for a2 in range(2):
                    for b2 in range(2):
                        src = bass.AP(o2.tensor,
                                      o2.offset + a2 * 180 + b2 * 90,
                                      [[o2.ap[0][0], 32], [2 * 180, nry3],
                                       [1, 90]])
                        nc.sync.dma_start(
                            out=ph3[(a2 * 2 + b2) * 32:(a2 * 2 + b2 + 1) * 32],
                            in_=src)
                T[s]["ph3"] = ph3
            for s, S in SG:
                nq, nt2, nt3 = S["nq"], S["nt2"], S["nt3"]
                ph3, mall = T[s]["ph3"], T[s]["mall"]
                dtc = bpool.tile([128, nt3, 92], dt.bfloat16, tag=f"dtc{s}",
                                 name=f"dtc{s}")
                nc.vector.memset(dtc[0:64, :, 0:2], 0.0)
                nc.vector.memset(dtc[0:64, :, 90:92], 0.0)
                m3 = bass.AP(mall.tensor, mall.offset + nt2,
                             [mall.ap[0], [1, nt3]])
                RPP3 = 4
                for t0 in range(0, nt3, RPP3):
                    nr = min(RPP3, nt3 - t0)
                    ps = ppool.tile([64, nr, 88], dt.float32, tag=f"ps{s}")
                    gi = 0
                    for dky in range(3):
                        for dmx in range(3):
                            g = dky * 3 + dmx
                            rhs = bass.AP(ph3.tensor,
                                          ph3.offset + (t0 + dky) * 90 + dmx,
                                          [ph3.ap[0], [90, nr], [1, 88]])
                            nc.tensor.matmul(ps[:], wt["w_dt3"][:, g, :], rhs,
                                             start=(gi == 0), stop=(gi == 8))
                            gi += 1
                    ev = wpool.tile([64, nr, 88], dt.bfloat16, tag=f"ev3{s}")
                    nc.scalar.activation(ev[:], ps[:], RELU,
                                         bias=ct["t_dt3"][0:64, 0:1],
                                         scale=ct["s_dt3"][0:64, 0:1])
                    mbb = bass.AP(m3.tensor, m3.offset + t0,
                                  [m3.ap[0], [1, nr], [0, 88]])
                    nc.vector.tensor_tensor(out=dtc[0:64, t0:t0 + nr, 2:90],
                                            in0=ev[:], in1=mbb[0:64],
                                            op=mybir.AluOpType.mult)
                # bottom half: rows shifted by one (ky=1 operand of the
                # packed dn1 matmul), copied via SBUF->SBUF DMA
                shsrc = bass.AP(dtc.tensor, dtc.offset + 92,
                                [[dtc.ap[0][0], 64], [92, nt3 - 1], [1, 92]])
                nc.sync.dma_start(out=dtc[64:128, 0:nt3 - 1, :], in_=shsrc)
                T[s]["dtc"] = dtc

            # ======== dn1 ========
            RPP = 5
            for s, S in SG:
                nq, nt2, nt3, nn1 = S["nq"], S["nt2"], S["nt3"], S["nn1"]
                xs, dtc, mall = T[s]["xs"], T[s]["dtc"], T[s]["mall"]
                mn1 = bass.AP(mall.tensor, mall.offset + nt2 + nt3,
                              [mall.ap[0], [1, nn1]])
                n1o = []
                for g in range(2):
                    t = bpool.tile([128, nn1, 92], dt.bfloat16,
                                   tag=f"n1o{g}_{s}", name=f"n1o{g}")
                    nc.vector.memset(t[:, :, 0:2], 0.0)
                    nc.vector.memset(t[:, :, 90:92], 0.0)
                    n1o.append(t)
                for ocg in range(2):
                    for r0 in range(0, nn1, RPP):
                        nr = min(RPP, nn1 - r0)
                        ps = ppool.tile([128, nr, 88], dt.float32, tag=f"ps{s}")
                        gi = 0
                        for ky in range(3):
                            for kx in range(3):
                                tap = ky * 3 + kx
                                for icc, srcT in enumerate((xs[0], xs[1])):
                                    rhs = bass.AP(
                                        srcT.tensor,
                                        srcT.offset + (r0 + ky + 1) * 92 + kx + 1,
                                        [srcT.ap[0], [92, nr], [1, 88]])
                                    lhs = wt["w_dn1"][:, tap * 3 + icc,
                                                      ocg * 128:(ocg + 1) * 128]
                                    nc.tensor.matmul(ps[:], lhs, rhs,
                                                     start=(gi == 0),
                                                     stop=(gi == 23))
                                    gi += 1
                        for kx in range(3):
                            # ky 0+1 packed: top = dtc row r+1, bottom = r+2
                            rhs = bass.AP(
                                dtc.tensor,
                                dtc.offset + (r0 + 1) * 92 + kx + 1,
                                [[dtc.ap[0][0], 128], [92, nr], [1, 88]])
                            nc.tensor.matmul(
                                ps[:], wt["w_dn1d"][:, kx,
                                                    ocg * 128:(ocg + 1) * 128],
                                rhs, start=False, stop=(gi == 23))
                            gi += 1
                            # ky=2 (K=64)
                            rhs = bass.AP(
                                dtc.tensor,
                                dtc.offset + (r0 + 3) * 92 + kx + 1,
                                [[dtc.ap[0][0], 64], [92, nr], [1, 88]])
                            nc.tensor.matmul(
                                ps[:], wt["w_dn1"][0:64, (6 + kx) * 3 + 2,
                                                   ocg * 128:(ocg + 1) * 128],
                                rhs, start=False, stop=(gi == 23))
                            gi += 1
                        ev = wpool.tile([128, nr, 88], dt.bfloat16,
                                        tag=f"evn1{s}")
                        nc.scalar.activation(ev[:], ps[:], RELU,
                                             bias=ct["t_dn1"][:, ocg:ocg + 1],
                                             scale=ct["s_dn1"][:, ocg:ocg + 1])
                        mbb = bass.AP(mn1.tensor, mn1.offset + r0,
                                      [mn1.ap[0], [1, nr], [0, 88]])
                        nc.vector.tensor_tensor(
                            out=n1o[ocg][:, r0:r0 + nr, 2:90],
                            in0=ev[:], in1=mbb, op=mybir.AluOpType.mult)
                T[s]["n1o"] = n1o

            # ==== dn2 (ACT direct write) then dn3 per segment, interleaved so
            # dn3(s0)'s scalar-engine work overlaps dn2(s1)'s matmuls ====
            feat_sb, depth_sb = {}, {}
            for s, S in SG:
                nout, nn1 = S["nout"], S["nn1"]
                n1o = T[s]["n1o"]
                n2o = []
                for g in range(2):
                    n2o.append(bpool.tile([128, nout, 88], dt.bfloat16,
                                          tag=f"n2o{g}_{s}", name=f"n2o{g}"))
                npix = nout * FW
                npc = (npix + 127) // 128
                feat_sb[s] = bpool.tile([128, npc, CIMG],
                                        dt.bfloat16, tag=f"feat{s}",
                                        name=f"feat_sb{s}")
                depth_sb[s] = bpool.tile([128, npc, DD],
                                         dt.bfloat16, tag=f"depth{s}",
                                         name=f"depth_sb{s}")
                n2f = [t.rearrange("p a b -> p (a b)") for t in n2o]

                def dn3_chunk(pc):
                    # dn3 + softmax (no max-sub; bias via K=1 matmul)
                    m = min(128, npix - pc * 128)
                    ps = ppool.tile([m, 139], dt.float32, tag=f"ps{s}")
                    for icc in range(2):
                        nc.tensor.matmul(ps[:], n2f[icc][:, pc * 128:pc * 128 + m],
                                         wt["w_dn3"][:, icc, :],
                                         start=(icc == 0), stop=False)
                    nc.tensor.matmul(ps[:], one1[:, 0:m], b3t[:],
                                     start=False, stop=True)
                    ex = wpool.tile([m, DD], dt.float32, tag=f"ex{s}")
                    sm = wpool.tile([m, 1], dt.float32, tag=f"sm{s}")
                    nc.scalar.activation(ex[:], ps[:, 0:DD],
                                         mybir.ActivationFunctionType.Exp,
                                         accum_out=sm[:, 0:1])
                    rc = wpool.tile([m, 1], dt.float32, tag=f"rc{s}")
                    nc.vector.reciprocal(rc[:], sm[:])
                    nc.vector.tensor_scalar(out=depth_sb[s][0:m, pc, :],
                                            in0=ex[:], scalar1=rc[:, 0:1],
                                            scalar2=None,
                                            op0=mybir.AluOpType.mult)
                    if pc % 2 == 0:
                        nc.scalar.activation(feat_sb[s][0:m, pc, :],
                                             ps[:, DD:DD + CIMG],
                                             mybir.ActivationFunctionType.Copy)
                    else:
                        nc.vector.tensor_copy(feat_sb[s][0:m, pc, :],
                                              ps[:, DD:DD + CIMG])

                done3 = 0
                for r0 in range(0, nout, RPP):
                    for ocg in range(2):
                        nr = min(RPP, nout - r0)
                        ps = ppool.tile([128, nr, 88], dt.float32, tag=f"ps{s}")
                        gi = 0
                        for ky in range(3):
                            for kx in range(3):
                                tap = ky * 3 + kx
                                for icc in range(2):
                                    rhs = bass.AP(
                                        n1o[icc].tensor,
                                        n1o[icc].offset + (r0 + ky) * 92 + kx + 1,
                                        [n1o[icc].ap[0], [92, nr], [1, 88]])
                                    lhs = wt["w_dn2"][:, tap * 2 + icc,
                                                      ocg * 128:(ocg + 1) * 128]
                                    nc.tensor.matmul(ps[:], lhs, rhs,
                                                     start=(gi == 0),
                                                     stop=(gi == 17))
                                    gi += 1
                        nc.scalar.activation(n2o[ocg][:, r0:r0 + nr, :], ps[:],
                                             RELU,
                                             bias=ct["t_dn2"][:, ocg:ocg + 1],
                                             scale=ct["s_dn2"][:, ocg:ocg + 1])
                    # emit dn3 chunks whose n2o rows are now complete
                    rows_done = min(r0 + RPP, nout)
                    while (done3 < npc
                           and -(-((done3 + 1) * 128) // 88) <= rows_done):
                        dn3_chunk(done3)
                        done3 += 1
                while done3 < npc:
                    dn3_chunk(done3)
                    done3 += 1
                T[s]["n2o"] = n2o
                # DMA outputs in SBUF-native [128, pc, d] layout (host
                # reorders); s0's overlap dn2(s1)/dn3(s1) compute
                nc.sync.dma_start(out=out_depth[s], in_=depth_sb[s][:])
                nc.sync.dma_start(out=out_feat[s], in_=feat_sb[s][:])
    nc.compile()
    return nc


# ------------------------------------------------------------ host helpers
def _host_geometry(rots, trans, intr, post_rots, post_trans):
    import jax
    import jax.numpy as jnp
    with jax.default_device(jax.devices("cpu")[0]):
        f32 = jnp.float32
        ds = jnp.arange(1.0, 60.0, 1.0, dtype=f32)
        xs = jnp.linspace(0.0, IW - 1.0, FW, dtype=f32)
        ys = jnp.linspace(0.0, IH - 1.0, FH, dtype=f32)
        dm = jnp.broadcast_to(ds[:, None, None], (DD, FH, FW))
        xm = jnp.broadcast_to(xs[None, None, :], (DD, FH, FW))
        ym = jnp.broadcast_to(ys[None, :, None], (DD, FH, FW))
        fr = jnp.stack([xm, ym, dm], -1)
        pts = fr[None, None] - jnp.asarray(post_trans)[:, :, None, None, None, :]
        pts = jnp.einsum("bnij,bndhwj->bndhwi",
                         jnp.linalg.inv(jnp.asarray(post_rots)), pts)
        pts = jnp.concatenate([pts[..., :2] * pts[..., 2:3], pts[..., 2:3]], -1)
        comb = jnp.einsum("bnij,bnjk->bnik", jnp.asarray(rots),
                          jnp.linalg.inv(jnp.asarray(intr)))
        pts = jnp.einsum("bnij,bndhwj->bndhwi", comb, pts) \
            + jnp.asarray(trans)[:, :, None, None, None, :]
        lo = jnp.array([XY0, XY0, Z0], dtype=f32)
        dxv = jnp.array([DXY, DXY, DZ], dtype=f32)
        g = ((pts - lo) / dxv).astype(jnp.int32).reshape(-1, 3)
        kept = ((g[:, 0] >= 0) & (g[:, 0] < NX) & (g[:, 1] >= 0) & (g[:, 1] < NX)
                & (g[:, 2] >= 0) & (g[:, 2] < NZ))
        flat = (g[:, 2] * NX + g[:, 0]) * NX + g[:, 1]
        return np.asarray(flat, np.int64), np.asarray(kept)


def _prep_a_inputs(inputs):
    """Build per-core input maps for launch A."""
    d = np.asarray(inputs["d"], np.float32).reshape(N, IH, IW)
    x_img = np.asarray(inputs["x_img"], np.float32)

    # dt1 folded affine: relu(alpha*d + beta), alpha = s*w, beta = s*b + t
    a1 = (inputs["dt1_s"] * inputs["dt1_w"][:, 0, 0, 0]).astype(np.float32)
    b1 = (inputs["dt1_s"] * inputs["dt1_b"] + inputs["dt1_t"]).astype(np.float32)
    cab = np.arange(128)
    dt1_alpha = a1[cab // 16][:, None]
    dt1_beta = b1[cab // 16][:, None]

    def wprep_dt2():
        w = np.asarray(inputs["dt2_w"], np.float32)      # [32,8,5,5]
        out = np.zeros((4, 128, 32), np.float32)
        for ky in range(5):
            for kx in range(5):
                a, dky = ky % 4, ky // 4
                bph, dmx = (kx + 2) % 4, (kx + 2) // 4
                g = dky * 2 + dmx
                rows = (np.arange(8)) * 16 + a * 4 + bph
                out[g, rows, :] = w[:, :, ky, kx].T
        return out.astype(bf16)

    def wprep_dt3():
        w = np.asarray(inputs["dt3_w"], np.float32)      # [64,32,5,5]
        out = np.zeros((9, 128, 64), np.float32)
        for ky in range(5):
            for kx in range(5):
                a, dky = ky % 2, ky // 2
                bph, dmx = kx % 2, (kx + 2) // 2 - 1
                g = dky * 3 + dmx
                rows = (a * 2 + bph) * 32 + np.arange(32)
                out[g, rows, :] = w[:, :, ky, kx].T
        return out.astype(bf16)

    def wprep_3x3(w, icc_sizes):
        O, I = w.shape[0], w.shape[1]
        nic = len(icc_sizes)
        out = np.zeros((9, nic, 128, O), np.float32)
        for ky in range(3):
            for kx in range(3):
                tap = ky * 3 + kx
                ic0 = 0
                for icc, sz in enumerate(icc_sizes):
                    out[tap, icc, 0:sz, :] = w[:, ic0:ic0 + sz, ky, kx].T
                    ic0 += sz
        return out.astype(bf16)

    # NOTE: dn1 input concat order is [dt3(64) | x_img(256)] in the reference;
    # our matmul chunks are (x0:128, x1:128, dt3:64) -> weight cols must match:
    w_dn1_full = np.asarray(inputs["dn1_w"], np.float32)
    w_dn1 = np.zeros((9, 3, 128, 256), np.float32)
    for ky in range(3):
        for kx in range(3):
            tap = ky * 3 + kx
            w_dn1[tap, 0, :, :] = w_dn1_full[:, 64:192, ky, kx].T
            w_dn1[tap, 1, :, :] = w_dn1_full[:, 192:320, ky, kx].T
            w_dn1[tap, 2, 0:64, :] = w_dn1_full[:, 0:64, ky, kx].T
    w_dn1 = w_dn1.astype(bf16)
    w_dn1d = np.zeros((3, 128, 256), np.float32)
    for kx in range(3):
        w_dn1d[kx, 0:64, :] = w_dn1_full[:, 0:64, 0, kx].T
        w_dn1d[kx, 64:128, :] = w_dn1_full[:, 0:64, 1, kx].T
    w_dn1d = w_dn1d.astype(bf16)
    w_dn2 = wprep_3x3(np.asarray(inputs["dn2_w"], np.float32), [128, 128])
    w_dn3 = np.asarray(inputs["dn3_w"], np.float32)[:, :, 0, 0]  # [139, 256]
    w_dn3p = np.zeros((2, 128, 139), np.float32)
    w_dn3p[0] = w_dn3[:, 0:128].T
    w_dn3p[1] = w_dn3[:, 128:256].T

    def fold_bias(b, s, t):
        # conv bias b then bn scale/shift: relu(s*(x+b) + t) = relu(s*x + (s*b+t))
        return np.asarray(s, np.float32), np.asarray(s * b + t, np.float32)

    s2, t2 = fold_bias(inputs["dt2_b"], inputs["dt2_s"], inputs["dt2_t"])
    s3, t3 = fold_bias(inputs["dt3_b"], inputs["dt3_s"], inputs["dt3_t"])
    sn1, tn1 = fold_bias(inputs["dn1_b"], inputs["dn1_s"], inputs["dn1_t"])
    sn2, tn2 = fold_bias(inputs["dn2_b"], inputs["dn2_s"], inputs["dn2_t"])
    b_dn3 = np.broadcast_to(np.asarray(inputs["dn3_b"], np.float32)[None, :],
                            (128, 139)).copy()

    consts = np.zeros((128, 153), np.float32)
    consts[:, 0] = dt1_alpha[:, 0]
    consts[:, 1] = dt1_beta[:, 0]
    consts[:, 2] = np.tile(s2, 4)
    consts[:, 3] = np.tile(t2, 4)
    consts[:, 4] = np.tile(s3, 2)
    consts[:, 5] = np.tile(t3, 2)
    consts[:, 6:8] = sn1.reshape(2, 128).T
    consts[:, 8:10] = tn1.reshape(2, 128).T
    consts[:, 10:12] = sn2.reshape(2, 128).T
    consts[:, 12:14] = tn2.reshape(2, 128).T
    consts[:, 14:153] = b_dn3
    shared = dict(
        consts=consts,
        w_dt2=wprep_dt2(), w_dt3=wprep_dt3(), w_dn1=w_dn1, w_dn1d=w_dn1d,
        w_dn2=w_dn2,
        w_dn3=w_dn3p.astype(bf16),
        bias_dn3=np.asarray(inputs["dn3_b"], np.float32)[None, :].astype(bf16),
    )
    # per-partition sentinel: alpha*sent + beta << 0 so relu gives exact 0
    sent_p = np.where(a1 > 0, -1e20, 1e20).astype(np.float32)[cab // 16]

    maps = []
    for c in range(NCORES):
        m = dict(shared)
        for s, (cam, h0) in enumerate([SEG_A[c], SEG_B[c]]):
            S = SEGS[s]
            d0 = 8 * h0 - 34
            dseg = np.zeros((S["nd"], 712), np.float32)
            lo, hi = max(0, d0), min(IH, d0 + S["nd"])
            if hi > lo:
                dseg[lo - d0:hi - d0, 4:708] = d[cam, lo:hi]
            padm = np.zeros((S["nd"], 712), bool)
            padm[:, 0:4] = True
            rows_g = d0 + np.arange(S["nd"])
            padm[(rows_g < 0) | (rows_g >= IH), :] = True
            nq = S["nq"]
            ph = dseg.reshape(nq, 4, 178, 4)[:, :, :177, :]     # ry a rx b
            ph = ph.transpose(1, 3, 0, 2)                        # a b ry rx
            phm = padm.reshape(nq, 4, 178, 4)[:, :, :177, :].transpose(1, 3, 0, 2)
            dphf = np.broadcast_to(
                ph[None], (8, 4, 4, nq, 177)).reshape(128, nq, 177)
            phmf = np.broadcast_to(
                phm[None], (8, 4, 4, nq, 177)).reshape(128, nq, 177)
            m[f"dph{s}"] = np.where(
                phmf, sent_p[:, None, None], dphf).astype(bf16)
            q0, t0, r0 = 2 * h0 - 8, h0 - 3, h0 - 1
            qr = np.arange(S["nt2"]) + q0
            m2m = np.broadcast_to(((qr >= 0) & (qr < 64))[None, :],
                                  (128, S["nt2"]))
            tr = np.arange(S["nt3"]) + t0
            m3m = np.broadcast_to(((tr >= 0) & (tr < FH))[None, :],
                                  (128, S["nt3"]))
            rr = np.arange(S["nn1"]) + r0
            mn1m = np.broadcast_to(((rr >= 0) & (rr < FH))[None, :],
                                   (128, S["nn1"]))
            m[f"masks{s}"] = np.concatenate(
                [m2m, m3m, mn1m], axis=1).astype(bf16)
            xseg = np.zeros((CIN, S["nt3"], 92), np.float32)
            lo2, hi2 = max(0, t0), min(FH, t0 + S["nt3"])
            if hi2 > lo2:
                xseg[:, lo2 - t0:hi2 - t0, 2:90] = x_img[cam, :, lo2:hi2, :]
            m[f"xseg{s}"] = xseg.astype(bf16)
        maps.append(m)
    return maps


# ------------------------------------------------------- launch B1 (h-runs)
# Pre-reduce depth*feat over image rows h: the 32 rows of one image column at
# one depth bin land in 1-3 distinct voxels ("runs"). A dense matmul with a
# host-built depth-weight matrix sums each run: contraction over h (K=32),
# lhsT[h, run] = depth, rhs[h, feat]. 4 columns share the 128 partitions.
NVCOL = 538                  # 528 (n,w) columns + splits for >128 runs
CG = 72                      # virtual columns per core (padded)
CGP = CG // 4                # free-dim slots (4 columns per partition group)
BPG = 6                      # columns per PSUM bank group


def build_launch_b1():
    nc = bacc.Bacc("TRN2", target_bir_lowering=False, debug=False,
                   num_devices=NCORES)
    lhsT = nc.dram_tensor("lhsT", [32, CG, 128], dt.bfloat16,
                          kind="ExternalInput").ap()
    featc = nc.dram_tensor("featc", [32, CG, CIMG], dt.bfloat16,
                           kind="ExternalInput").ap()
    msum = nc.dram_tensor("msum", [128, CG, CIMG], dt.bfloat16,
                          kind="ExternalOutput").ap()
    with tile.TileContext(nc) as tc:
        with tc.tile_pool(name="const", bufs=1) as cpool, \
             tc.tile_pool(name="ps", bufs=8, space="PSUM") as pp:
            H1 = 3 * BPG                # first split: 3 groups
            lts, fts = [], []
            for hi, (a, b) in enumerate([(0, H1), (H1, CG)]):
                ltc = cpool.tile([32, b - a, 128], dt.bfloat16, name=f"lt{hi}")
                ftc = cpool.tile([32, b - a, CIMG], dt.bfloat16, name=f"ft{hi}")
                nc.sync.dma_start(out=ltc[:], in_=lhsT[:, a:b, :])
                nc.sync.dma_start(out=ftc[:], in_=featc[:, a:b, :])
                lts.append(ltc)
                fts.append(ftc)
            ot = cpool.tile([128, CG, CIMG], dt.bfloat16, name="ot")
            ngrp = CG // BPG
            for gi in range(ngrp):           # K=32 matmul per column, base 0
                g = gi * BPG
                ps = pp.tile([128, BPG, CIMG], dt.float32, tag="ps", name="ps")
                for b in range(BPG):
                    hi = 0 if g + b < H1 else 1
                    off = 0 if hi == 0 else H1
                    nc.tensor.matmul(ps[:, b, :],
                                     lts[hi][:, g + b - off, :],
                                     fts[hi][:, g + b - off, :],
                                     start=True, stop=True)
                # alternate PSUM drain between scalar and vector engines
                if gi % 2 == 0:
                    nc.scalar.activation(ot[:, g:g + BPG, :], ps[:],
                                         mybir.ActivationFunctionType.Copy)
                else:
                    nc.vector.tensor_copy(ot[:, g:g + BPG, :], ps[:])
                if gi in (3, 7, 10, 11):
                    g0 = {3: 0, 7: 4, 10: 8, 11: 11}[gi] * BPG
                    nc.sync.dma_start(out=msum[:, g0:g + BPG, :],
                                      in_=ot[:, g0:g + BPG, :])
    nc.compile()
    return nc


# --------------------------------------------------- launch B2 (voxel scatter)
def build_launch_b2(W):
    """W single-chunk windows of <=128 superpoints (voxel span < 128);
    one-hot scatter-sum, 4 windows per PSUM bank."""
    nc = bacc.Bacc("TRN2", target_bir_lowering=False, debug=False,
                   num_devices=NCORES)
    pb = nc.dram_tensor("pb", [128, W, CIMG], dt.bfloat16,
                        kind="ExternalInput").ap()
    offv = nc.dram_tensor("offv", [128, W], dt.float32,
                          kind="ExternalInput").ap()

    povirt = nc.dram_tensor("povirt", [128, W, CIMG], dt.bfloat16,
                            kind="ExternalOutput").ap()
    with tile.TileContext(nc) as tc:
        with tc.tile_pool(name="const", bufs=1) as cpool, \
             tc.tile_pool(name="g", bufs=12) as gp, \
             tc.tile_pool(name="ps", bufs=8, space="PSUM") as pp:
            iota4 = cpool.tile([128, 1, 128], dt.bfloat16, name="iota4")
            nc.gpsimd.iota(iota4[:], pattern=[[0, 1], [1, 128]], base=0,
                           channel_multiplier=0,
                           allow_small_or_imprecise_dtypes=True)
            offt = cpool.tile([128, W], dt.float32, name="offt")
            nc.sync.dma_start(out=offt[:], in_=offv)
            pbt = cpool.tile([128, W, CIMG], dt.bfloat16, name="pbt")
            qch = (W + 3) // 4
            for q0 in range(0, W, qch):
                q1 = min(q0 + qch, W)
                nc.sync.dma_start(out=pbt[:, q0:q1, :], in_=pb[:, q0:q1, :])

            ot = cpool.tile([128, W, CIMG], dt.bfloat16, name="ot")
            ps = None
            out_done = 0
            for w in range(W):
                if w % 4 == 0:
                    ps = pp.tile([128, 4, CIMG], dt.float32, tag="ps",
                                 name="ps")
                g = gp.tile([128, 128], dt.bfloat16, tag="g", name="g")
                nc.vector.tensor_scalar(
                    out=g[:], in0=iota4[:, 0, :], scalar1=offt[:, w:w + 1],
                    scalar2=None, op0=mybir.AluOpType.is_equal)
                nc.tensor.matmul(ps[:, w % 4, :], g[:], pbt[:, w, :],
                                 start=True, stop=True)
                if w % 4 == 3 or w == W - 1:
                    n4 = w % 4 + 1
                    w0 = w - n4 + 1
                    nc.scalar.activation(ot[:, w0:w + 1, :],
                                         ps[:, 0:n4, :],
                                         mybir.ActivationFunctionType.Copy)
                flush = (w == W - 1 or
                         ((w + 1) % 16 == 0 if w < W - 12 else (w + 1) % 4 == 0))
                if flush:
                    nc.sync.dma_start(out=povirt[:, out_done:w + 1, :],
                                      in_=ot[:, out_done:w + 1, :])
                    out_done = w + 1
    nc.compile()
    return nc


# ---------------------------------------------------------------- launch C
C_OUT_ROWS = 23              # ds2-out rows per core (8*23 = 184 >= 180)


def build_launch_c():
    nc = bacc.Bacc("TRN2", target_bir_lowering=False, debug=False,
                   num_devices=NCORES)
    NR1 = C_OUT_ROWS + 2                         # ds1-out rows incl halo (25)
    NRP = 2 * NR1 + 1                            # pooled rows needed (51)
    slab = nc.dram_tensor("slab", [CIMG, NRP, 362], dt.bfloat16,
                          kind="ExternalInput").ap()
    m1 = nc.dram_tensor("m1", [128, NR1], dt.bfloat16, kind="ExternalInput").ap()
    wd1 = nc.dram_tensor("wd1", [9, CIMG, CIMG], dt.bfloat16,
                         kind="ExternalInput").ap()
    wd2 = nc.dram_tensor("wd2", [9, CIMG, CIMG], dt.bfloat16,
                         kind="ExternalInput").ap()
    sb1 = nc.dram_tensor("sb1", [CIMG, 2], dt.float32, kind="ExternalInput").ap()
    sb2 = nc.dram_tensor("sb2", [CIMG, 2], dt.float32, kind="ExternalInput").ap()
    yout = nc.dram_tensor("yout", [CIMG, C_OUT_ROWS, 180], dt.bfloat16,
                          kind="ExternalOutput").ap()
    RELU = mybir.ActivationFunctionType.Relu
    RP = 2
    with tile.TileContext(nc) as tc:
        with tc.tile_pool(name="const", bufs=1) as cpool, \
             tc.tile_pool(name="work", bufs=3) as wp, \
             tc.tile_pool(name="big", bufs=1) as bp, \
             tc.tile_pool(name="ps", bufs=4, space="PSUM") as pp:
            slabt = bp.tile([CIMG, NRP, 362], dt.bfloat16, name="slabt")
            w1 = cpool.tile([CIMG, 9, CIMG], dt.bfloat16, name="w1")
            nc.sync.dma_start(out=w1[:], in_=wd1.rearrange("t p o -> p t o"))
            sb1t = cpool.tile([CIMG, 2], dt.float32, name="sb1t")
            nc.sync.dma_start(out=sb1t[:], in_=sb1)
            m1t = wp.tile([128, NR1], dt.bfloat16, name="m1t")
            nc.sync.dma_start(out=m1t[:], in_=m1)
            nc.sync.dma_start(out=slabt[:, 0:5, :], in_=slab[:, 0:5, :])
            nc.sync.dma_start(out=slabt[:, 5:11, :], in_=slab[:, 5:11, :])
            nc.sync.dma_start(out=slabt[:, 11:19, :], in_=slab[:, 11:19, :])
            w2 = cpool.tile([CIMG, 9, CIMG], dt.bfloat16, name="w2")
            nc.sync.dma_start(out=w2[:], in_=wd2.rearrange("t p o -> p t o"))
            sb2t = cpool.tile([CIMG, 2], dt.float32, name="sb2t")
            nc.sync.dma_start(out=sb2t[:], in_=sb2)
            for rr in range(19, NRP, 8):
                nrr = min(8, NRP - rr)
                nc.sync.dma_start(out=slabt[:, rr:rr + nrr, :],
                                  in_=slab[:, rr:rr + nrr, :])
            h1 = bp.tile([CIMG, NR1, 182], dt.bfloat16, name="h1")
            nc.vector.memset(h1[:, :, 0:1], 0.0)
            nc.vector.memset(h1[:, :, 181:182], 0.0)
            yo = bp.tile([CIMG, C_OUT_ROWS, 180], dt.bfloat16, name="yo")

            def ds1_group(t0):
                # ds1: stride-2 3x3; out row t reads slab rows 2t..2t+2
                nr = min(RP, NR1 - t0)
                ps = pp.tile([CIMG, nr, 180], dt.float32, tag="ps1", name="ps")
                gi = 0
                for ky in range(3):
                    for kx in range(3):
                        rhs = bass.AP(slabt.tensor,
                                      slabt.offset + (2 * t0 + ky) * 362 + kx,
                                      [slabt.ap[0], [2 * 362, nr], [2, 180]])
                        nc.tensor.matmul(ps[:], w1[:, ky * 3 + kx, :], rhs,
                                         start=(gi == 0), stop=(gi == 8))
                        gi += 1
                ev = wp.tile([CIMG, nr, 180], dt.bfloat16, tag="ev", name="ev")
                nc.scalar.activation(ev[:], ps[:], RELU, bias=sb1t[:, 1:2],
                                     scale=sb1t[:, 0:1])
                mbb = bass.AP(m1t.tensor, m1t.offset + t0,
                              [[m1t.ap[0][0], CIMG], [1, nr], [0, 180]])
                nc.vector.tensor_tensor(out=h1[:, t0:t0 + nr, 1:181],
                                        in0=ev[:], in1=mbb,
                                        op=mybir.AluOpType.mult)

            def ds2_group(o0):
                # ds2: 3x3 pad 1: out row o reads h1 rows o..o+2
                nr = min(RP, C_OUT_ROWS - o0)
                ps = pp.tile([CIMG, nr, 180], dt.float32, tag="ps2", name="ps")
                gi = 0
                for ky in range(3):
                    for kx in range(3):
                        rhs = bass.AP(h1.tensor,
                                      h1.offset + (o0 + ky) * 182 + kx,
                                      [h1.ap[0], [182, nr], [1, 180]])
                        nc.tensor.matmul(ps[:], w2[:, ky * 3 + kx, :], rhs,
                                         start=(gi == 0), stop=(gi == 8))
                        gi += 1
                nc.scalar.activation(yo[:, o0:o0 + nr, :], ps[:], RELU,
                                     bias=sb2t[:, 1:2], scale=sb2t[:, 0:1])
                gidx = o0 // RP
                if gidx in (3, 7, 9, 11) or o0 + nr >= C_OUT_ROWS:
                    y0 = {3: 0, 7: 8, 9: 16, 11: 20}.get(gidx, 20)
                    nc.sync.dma_start(out=yout[:, y0:o0 + nr, :],
                                      in_=yo[:, y0:o0 + nr, :])

            # interleave: ds2 group o0 needs h1 rows o0..o0+3, i.e. ds1
            # groups through (o0+3)//2
            done2 = 0
            for t0 in range(0, NR1, RP):
                ds1_group(t0)
                rows_done = min(t0 + RP, NR1)
                while done2 < C_OUT_ROWS and done2 + 4 <= rows_done:
                    ds2_group(done2)
                    done2 += RP
            while done2 < C_OUT_ROWS:
                ds2_group(done2)
                done2 += RP
    nc.compile()
    return nc


_CACHE = {}


def run_launch_a(inputs):
    if "A" not in _CACHE:
        _CACHE["A"] = build_launch_a()
    nc = _CACHE["A"]
    maps = _prep_a_inputs(inputs)
    res = run_bass_kernel_spmd(nc, maps, list(range(NCORES)))
    depth = np.zeros((NPIX, DD), np.float32)
    feat = np.zeros((NPIX, CIMG), np.float32)
    for c in range(NCORES):
        r = res.results[c]
        for s, (cam, h0) in enumerate([SEG_A[c], SEG_B[c]]):
            S = SEGS[s]
            npix = S["nout"] * FW
            base = (cam * FH + h0) * FW
            # device layout [128, pc, d]: pixel = pc*128 + p
            dseg = r[f"out_depth{s}"].astype(np.float32).transpose(
                1, 0, 2).reshape(-1, DD)
            fseg = r[f"out_feat{s}"].transpose(1, 0, 2).reshape(-1, CIMG)
            depth[base:base + npix] = dseg[0:npix]
            feat[base:base + npix] = fseg[0:npix].astype(np.float32)
    return depth, feat


def _build_runs(flat, kept):
    """Extract h-runs (superpoints): maximal runs of consecutive image rows h
    within one (camera, depth-bin, column) group mapping to the same voxel.
    Geometry-only (no device data needed). Returns run table + vcol layout."""
    vox4 = flat.reshape(N, DD, FH, FW)
    kept4 = kept.reshape(N, DD, FH, FW)
    # reorder to [(n,w), d, h]
    v = np.ascontiguousarray(
        np.where(kept4, vox4, -1).transpose(0, 3, 1, 2)).reshape(
            N * FW, DD, FH)
    chg = np.ones(v.shape, bool)
    chg[:, :, 1:] = v[:, :, 1:] != v[:, :, :-1]
    st = chg & (v >= 0)
    sp_col, sp_d, sp_h0 = np.where(st)                 # sorted (col, d, h)
    nsp = sp_col.size
    stf = st.reshape(-1)
    vf = v.reshape(-1)
    rid = np.cumsum(stf) - 1
    sp_len = np.bincount(rid[vf >= 0], minlength=nsp)
    sp_vox = vf[stf]

    # virtual columns: split columns with >128 runs
    runs_per_col = np.bincount(sp_col, minlength=N * FW)
    col_start = np.concatenate([[0], np.cumsum(runs_per_col)])
    vcols = []                                         # (col, sp_start, count)
    for col in range(N * FW):
        s, c = int(col_start[col]), int(runs_per_col[col])
        while c > 128:
            vcols.append((col, s, 128))
            s += 128
            c -= 128
        vcols.append((col, s, c))
    assert len(vcols) <= NCORES * CG, f"{len(vcols)} vcols > {NCORES * CG}"
    # assign contiguous blocks of vcols to cores; per-sp (core1, slot, j)
    sp_core1 = np.zeros(nsp, np.int32)
    sp_slot = np.zeros(nsp, np.int32)
    sp_j = np.zeros(nsp, np.int32)
    percore = (len(vcols) + NCORES - 1) // NCORES
    vassign = []                                       # per core: list of vcols
    for c in range(NCORES):
        vassign.append(vcols[c * percore:(c + 1) * percore])
        for sl, (col, s, cnt) in enumerate(vassign[-1]):
            sp_core1[s:s + cnt] = c
            sp_slot[s:s + cnt] = sl
            sp_j[s:s + cnt] = np.arange(cnt)
    return dict(nsp=nsp, col=sp_col, d=sp_d, h0=sp_h0, ln=sp_len, vox=sp_vox,
                core1=sp_core1, slot=sp_slot, j=sp_j, vassign=vassign)


def _prep_b1_inputs(runs, depth_rows, feat_rows):
    """Fill lhsT depth weights + per-column feature tiles from launch-A out."""
    depth4 = depth_rows.reshape(N, FH, FW, DD)
    feat4 = feat_rows.reshape(N, FH, FW, CIMG)
    nsp, ln = runs["nsp"], runs["ln"]
    # expand runs to points
    tot = int(ln.sum())
    pt_rid = np.repeat(np.arange(nsp), ln)
    cum = np.concatenate([[0], np.cumsum(ln)])[:-1]
    pt_h = np.arange(tot) - np.repeat(cum, ln) + np.repeat(runs["h0"], ln)
    cam = runs["col"] // FW
    wim = runs["col"] % FW
    dvals = depth4[cam[pt_rid], pt_h, wim[pt_rid], runs["d"][pt_rid]]
    lhsT = np.zeros((NCORES, 32, CG, 128), bf16)
    lhsT[runs["core1"][pt_rid], pt_h, runs["slot"][pt_rid],
         runs["j"][pt_rid]] = dvals
    featc = np.zeros((NCORES, 32, CG, CIMG), bf16)
    for c in range(NCORES):
        for sl, (col, s, cnt) in enumerate(runs["vassign"][c]):
            featc[c, :, sl, :] = feat4[col // FW, :, col % FW, :].astype(bf16)
    return [dict(lhsT=lhsT[c], featc=featc[c]) for c in range(NCORES)]


def _build_schedule2(runs):
    """Shard superpoints by voxel x-row (greedy), sort by local voxel, emit
    single-chunk windows of <=128 superpoints with vox-span < 128."""
    sp_vox = runs["vox"]
    vx = (sp_vox // NX).astype(np.int64)
    cnt = np.bincount(vx, minlength=NX)
    order = np.argsort(-cnt, kind="stable")
    core_of_row = np.zeros(NX, np.int32)
    load = np.zeros(NCORES, np.int64)
    for r in order:
        c = int(np.argmin(load))
        core_of_row[r] = c
        load[c] += cnt[r]
    row_rank = np.zeros(NX, np.int64)
    rows_of = []
    for c in range(NCORES):
        rows = np.where(core_of_row == c)[0]
        rows_of.append(rows)
        row_rank[rows] = np.arange(len(rows))
    schedules = []
    for c in range(NCORES):
        sel = np.where(core_of_row[vx] == c)[0]        # superpoint ids
        vloc = row_rank[vx[sel]] * NX + (sp_vox[sel] % NX)
        o = np.argsort(vloc, kind="stable")
        sel, vloc = sel[o], vloc[o]
        win = []
        i, n = 0, len(sel)
        while i < n:
            base = vloc[i]
            j = min(i + 128, n)
            hi = np.searchsorted(vloc, base + 128, "left")
            j = min(j, hi)
            win.append((i, j, base))
            i = j
        schedules.append(dict(sel=sel, vloc=vloc, win=win, rows=rows_of[c]))
    W = max(len(s["win"]) for s in schedules)
    W = (W + 3) // 4 * 4
    return schedules, W


def _prep_b2_inputs(schedules, W, msums, runs):
    """Gather superpoint values (from B1 outputs) into window slots."""
    spc1, spsl, spj = runs["core1"], runs["slot"], runs["j"]
    maps = []
    NHG = (W + 1) // 2
    pp = np.arange(128)
    for sch in schedules:
        pb = np.zeros((128, W, CIMG), bf16)
        offv = np.full((128, W), -1.0, np.float32)
        sel, vloc = sch["sel"], sch["vloc"]
        for w, (i, j, base) in enumerate(sch["win"]):
            L = j - i
            s = sel[i:j]
            pb[0:L, w] = msums[spc1[s], spj[s], spsl[s]]
            offv[0:L, w] = vloc[i:j] - base
        hostg = np.zeros((128, NHG, 128), bf16)
        for w in range(0, W, 2):
            off = offv[:, w]
            v = off >= 0
            hostg[pp[v], w // 2, off[v].astype(np.int64)] = 1.0
        maps.append(dict(pb=pb, offv=offv, hostg=hostg))
    return maps


def _prep_c_inputs(inputs, pooled_t):
    """pooled_t: [CIMG, 360, 360] f32 -> per-core slabs + masks + weights."""
    NR1 = C_OUT_ROWS + 2
    NRP = 2 * NR1 + 1
    w1 = np.asarray(inputs["ds1_w"], np.float32)
    w2 = np.asarray(inputs["ds2_w"], np.float32)
    wd1 = np.stack([w1[:, :, ky, kx].T for ky in range(3) for kx in range(3)])
    wd2 = np.stack([w2[:, :, ky, kx].T for ky in range(3) for kx in range(3)])
    sb1 = np.stack([np.asarray(inputs["ds1_s"], np.float32),
                    np.asarray(inputs["ds1_t"], np.float32)], 1)
    sb2 = np.stack([np.asarray(inputs["ds2_s"], np.float32),
                    np.asarray(inputs["ds2_t"], np.float32)], 1)
    shared = dict(wd1=wd1.astype(bf16), wd2=wd2.astype(bf16), sb1=sb1, sb2=sb2)
    maps = []
    pt_bf = pooled_t.astype(bf16)
    for c in range(NCORES):
        o0g = C_OUT_ROWS * c
        p0 = 2 * o0g - 3
        slab = np.zeros((CIMG, NRP, 362), bf16)
        lo, hi = max(0, p0), min(NX, p0 + NRP)
        if hi > lo:
            slab[:, lo - p0:hi - p0, 1:361] = pt_bf[:, lo:hi, :]
        t1g = np.arange(NR1) + (o0g - 1)
        m1 = np.broadcast_to(((t1g >= 0) & (t1g < 180))[None, :],
                             (128, NR1)).astype(bf16)
        maps.append(dict(shared, slab=slab, m1=np.ascontiguousarray(m1)))
    return maps


def kernel(**inputs):
    inputs = {k: np.asarray(v) for k, v in inputs.items()}
    flat, kept = _host_geometry(inputs["cam2lidar_rots"],
                                inputs["cam2lidar_trans"], inputs["intrins"],
                                inputs["post_rots"], inputs["post_trans"])
    runs = _build_runs(flat, kept)
    schedules, W = _build_schedule2(runs)

    depth_rows, feat_rows = run_launch_a(inputs)

    if "B1" not in _CACHE:
        _CACHE["B1"] = build_launch_b1()
    b1maps = _prep_b1_inputs(runs, depth_rows, feat_rows)
    res_b1 = run_bass_kernel_spmd(_CACHE["B1"], b1maps, list(range(NCORES)))
    msums = np.stack([res_b1.results[c]["msum"] for c in range(NCORES)])

    key = ("B2", W)
    if key not in _CACHE:
        _CACHE[key] = build_launch_b2(W)
    b2maps = _prep_b2_inputs(schedules, W, msums, runs)
    res_b = run_bass_kernel_spmd(_CACHE[key], b2maps, list(range(NCORES)))

    pooled = np.zeros((NX * NX, CIMG), np.float32)
    for c, sch in enumerate(schedules):
        virt = res_b.results[c]["povirt"].astype(
            np.float32).transpose(1, 0, 2)                    # -> [W, 128, C]
        rows_arr = sch["rows"]
        nloc = len(rows_arr) * NX
        for w, (i, j, base) in enumerate(sch["win"]):
            span = min(128, nloc - base)
            lidx = base + np.arange(span)
            ridx = rows_arr[lidx // NX] * NX + (lidx % NX)
            pooled[ridx] += virt[w][:span]
    pooled_t = np.ascontiguousarray(
        pooled.reshape(NX, NX, CIMG).transpose(2, 0, 1))

    if "C" not in _CACHE:
        _CACHE["C"] = build_launch_c()
    cmaps = _prep_c_inputs(inputs, pooled_t)
    res_c = run_bass_kernel_spmd(_CACHE["C"], cmaps, list(range(NCORES)))
    out = np.zeros((1, CIMG, 180, 180), np.float32)
    for c in range(NCORES):
        o0g = C_OUT_ROWS * c
        nr = min(C_OUT_ROWS, 180 - o0g)
        if nr > 0:
            out[0, :, o0g:o0g + nr, :] = res_c.results[c]["yout"][:, 0:nr, :].astype(np.float32)
    return out

